# revision 15
# baseline (speedup 1.0000x reference)
"""ContextSNN (2-layer LIF spiking net, T=50) on 8 Trainium2 NeuronCores.

Strategy:
  - Data-parallel: batch B=4096 sharded 512 per core; weights replicated.
  - fc1 is a SINGLE fp16 matmul pass per step with mean-centered operands:
    host ships xc = fp16(x - 0.5). Centering halves both the x-rounding
    residual and the |x| magnitude multiplying the W1-rounding residual,
    so one fp16 pass lands within the spike-count error tolerance
    (the LIF dynamics are mildly chaotic; rel err ~1.5e-2 vs the 2e-2
    gate, deterministic). fp8 in any operand fails the gate (tested via
    exact host emulation: >=5.5e-2), as does fp16 LIF state (3.2e-2), so
    fp16-matmul + fp32-LIF numerics are locked; the kernel is engineered
    to sit at the fp16 PE roofline.
  - Engine split per step so the PE is the only bottleneck at any clock:
      PE    : 48 fc1 matmuls + 4 fc2 matmuls (fp16, N=512)
      DVE   : 5x PSUM-consuming scalar_tensor_tensor (reset+cur merge)
      GPSIMD: 5x leak update u = beta*u + t1 (SBUF-only) + acc add
      ACT   : 5x spike via Sign(u - thr) -> +-1 spikes (exact constant
              folds: W2/2, reset scalar -THR/2, output (raw+50)/2)
  - Layer-1 spikes +-1 fp16; every constant drift folds exactly
    (float64 on host) into per-neuron threshold/init constants:
      C1 = b1 + 0.5*sum(W1,in) - THR/2,  thr1 = THR - C1/(1-beta)
      C2 = b2 + 0.5*sum(W2,h) - THR/2,   thr2 = THR - C2/(1-beta)
  - 44 warmup matmuls on a zeroed tile keep the PE HAM clock gate at
    full rate through the initial DMA window (measured 3.2us re-throttle
    gap with fewer).
  - fc2 matmuls of step t-1 are slotted behind step t's first fc1 block
    so the PE never stalls on the vector-engine LIF chain.
"""
import sys
sys.path.insert(0, "/opt/trn_rl_repo")
import numpy as np
from contextlib import ExitStack

import concourse.bass as bass
import concourse.tile as tile
from concourse import bacc, mybir
from concourse.bass_utils import run_bass_kernel_spmd

T, B, IN, H, OUT = 50, 4096, 1500, 512, 45
OUTP = 64           # OUT padded on device (partial-partition matmul penalty)
INP = 1536          # IN padded to 12*128
NCORES = 8
BS = B // NCORES    # 512
BETA, THR = 0.9, 1.0
KT1 = INP // 128    # 12
MT1 = H // 128      # 4
KT2 = H // 128      # 4
XCHUNK = 4          # fc1 k-chunks per xc DMA (fewer, bigger transfers)
XB = KT1 // XCHUNK  # 3 DMA batches per step
WARM_MMS = 20
SPIKE_ACT = True    # spike on scalar engine (Sign, +-1) vs DVE is_gt (+-0.5)
OP2_GPS = False     # leak update on gpsimd vs DVE (Pool lacks the STT opcode)
ACC_GPS = False     # acc add on gpsimd (shares the DVE SBUF port; slows DVE)
FC2_AFTER_M = 2     # emit fc2 after this fc1 block (slack for the spike chain)
SCODE = 1.0 if SPIKE_ACT else 0.5   # spike magnitude on device
f16 = mybir.dt.float16
f32 = mybir.dt.float32
ALU = mybir.AluOpType
ACT_F = mybir.ActivationFunctionType

_NC_CACHE = {}


def _build():
    if "nc" in _NC_CACHE:
        return _NC_CACHE["nc"]
    nc = bacc.Bacc("TRN2", target_bir_lowering=False, debug=False, num_devices=NCORES)

    xc_d = nc.dram_tensor("xc", [T, XB, 128, XCHUNK * BS], f16, kind="ExternalInput").ap()
    xc0_d = nc.dram_tensor("xc0", [XCHUNK, 128, BS], f16, kind="ExternalInput").ap()
    w1h_d = nc.dram_tensor("w1h", [INP, H], f16, kind="ExternalInput").ap()
    w2h_d = nc.dram_tensor("w2h", [H, OUTP], f16, kind="ExternalInput").ap()
    nthr1_d = nc.dram_tensor("nthr1", [128, MT1], f32, kind="ExternalInput").ap()
    thr1p_d = nc.dram_tensor("thr1p", [128, MT1], f32, kind="ExternalInput").ap()
    u1i_d = nc.dram_tensor("u1i", [128, MT1], f32, kind="ExternalInput").ap()
    nthr2_d = nc.dram_tensor("nthr2", [OUTP, 1], f32, kind="ExternalInput").ap()
    thr2p_d = nc.dram_tensor("thr2p", [OUTP, 1], f32, kind="ExternalInput").ap()
    u2i_d = nc.dram_tensor("u2i", [OUTP, 1], f32, kind="ExternalInput").ap()
    out_d = nc.dram_tensor("out", [OUTP, BS], f32, kind="ExternalOutput").ap()

    op2_eng = nc.gpsimd if OP2_GPS else nc.vector

    with tile.TileContext(nc) as tc:
        with ExitStack() as ctx:
            wpool = ctx.enter_context(tc.tile_pool(name="w", bufs=1))
            xcpool = ctx.enter_context(tc.tile_pool(name="xc", bufs=18))
            state = ctx.enter_context(tc.tile_pool(name="state", bufs=1))
            spk1pool = ctx.enter_context(tc.tile_pool(name="spk1", bufs=2))
            spk2pool = ctx.enter_context(tc.tile_pool(name="spk2", bufs=2))
            t1pool = ctx.enter_context(tc.tile_pool(name="t1", bufs=8))
            t2pool = ctx.enter_context(tc.tile_pool(name="t2", bufs=2))
            ps1 = ctx.enter_context(tc.tile_pool(name="ps1", bufs=6, space="PSUM"))
            ps2 = ctx.enter_context(tc.tile_pool(name="ps2", bufs=2, space="PSUM"))

            # small state-constant DMAs go first on the gpsimd queue so the
            # state inits (which FIFO-queue head-of-line block) aren't stuck
            # behind ~1.6MB of weight DMAs.
            w1h0 = wpool.tile([128, H], f16, tag="w1h0")
            nc.gpsimd.dma_start(w1h0[:], w1h_d[0:128, :])
            w1h_t = [w1h0]
            nthr1 = wpool.tile([128, MT1], f32, tag="nthr1")
            thr1p = wpool.tile([128, MT1], f32, tag="thr1p")
            u1i = wpool.tile([128, MT1], f32, tag="u1i")
            nc.gpsimd.dma_start(nthr1[:], nthr1_d[:])
            nc.gpsimd.dma_start(thr1p[:], thr1p_d[:])
            nc.gpsimd.dma_start(u1i[:], u1i_d[:])
            nthr2 = wpool.tile([OUTP, 1], f32, tag="nthr2")
            thr2p = wpool.tile([OUTP, 1], f32, tag="thr2p")
            u2i = wpool.tile([OUTP, 1], f32, tag="u2i")
            nc.gpsimd.dma_start(nthr2[:], nthr2_d[:])
            nc.gpsimd.dma_start(thr2p[:], thr2p_d[:])
            nc.gpsimd.dma_start(u2i[:], u2i_d[:])

            # dummy matmuls on a zeroed tile, emitted before the state inits
            # so they start immediately: warm the PE HAM clock gate during
            # the initial DMA wait and bridge until the first real matmul
            # (psum never read).
            warm = state.tile([128, BS], f16, tag="warm")
            nc.vector.memset(warm[:], 0.0)
            for _ in range(WARM_MMS):
                pw = ps1.tile([128, BS], f32, tag="p1")
                nc.tensor.matmul(pw[:], warm[:, 0:128], warm[:], start=True, stop=True)
            if SPIKE_ACT:
                # pre-trigger the ACT spline-table load off the critical path
                warm_s = state.tile([128, 1], f16, tag="warms")
                nc.scalar.activation(warm_s[:], warm[:, 0:1], ACT_F.Sign, bias=0.0)

            # one tile per k-chunk: the first matmul only waits on the k=0
            # DMA instead of all of them (tile-granular dependencies).
            for k in range(1, KT1):
                wht = wpool.tile([128, H], f16, tag=f"w1h{k}")
                nc.gpsimd.dma_start(wht[:], w1h_d[k * 128:(k + 1) * 128, :])
                w1h_t.append(wht)
            w2h = wpool.tile([128, KT2 * OUTP], f16, tag="w2h")
            for k in range(KT2):
                nc.gpsimd.dma_start(w2h[:, k * OUTP:(k + 1) * OUTP], w2h_d[k * 128:(k + 1) * 128, :])
            zeros1 = state.tile([128, BS], f32, tag="zeros1")
            nc.vector.memset(zeros1[:], 0.0)
            u1 = state.tile([128, MT1 * BS], f32, tag="u1")
            for m in range(MT1):
                nc.vector.tensor_scalar(
                    u1[:, m * BS:(m + 1) * BS], zeros1[:], u1i[:, m:m + 1], None, ALU.add
                )
            zeros2 = state.tile([OUTP, BS], f32, tag="zeros2")
            nc.vector.memset(zeros2[:], 0.0)
            u2 = state.tile([OUTP, BS], f32, tag="u2")
            nc.vector.tensor_scalar(u2[:], zeros2[:], u2i[:, 0:1], None, ALU.add)
            acc = state.tile([OUTP, BS], f32, tag="acc")
            nc.vector.memset(acc[:], 0.0)

            # layer-1 spikes stored as +-SCODE: -SCODE = "no spike"
            spk1_prev = spk1pool.tile([128, MT1 * BS], f16)
            nc.vector.memset(spk1_prev[:], -SCODE)
            spk2_prev = spk2pool.tile([OUTP, BS], f16)
            nc.vector.memset(spk2_prev[:], -SCODE)

            def emit_fc2_mms(spk1_t):
                p2 = ps2.tile([OUTP, BS], f32)
                for k in range(KT2):
                    ksl = slice(k * OUTP, (k + 1) * OUTP)
                    bsl = slice(k * BS, (k + 1) * BS)
                    nc.tensor.matmul(p2[:], w2h[:, ksl], spk1_t[:, bsl],
                                     start=(k == 0), stop=(k == KT2 - 1))
                return p2

            def emit_spike(dst, src, nthr_ap, thr_ap):
                if SPIKE_ACT:
                    nc.scalar.activation(dst, src, ACT_F.Sign, bias=nthr_ap)
                else:
                    nc.vector.tensor_scalar(dst, src, thr_ap, 0.5,
                                            ALU.is_gt, ALU.subtract)

            def emit_lif2(p2, spk2_p):
                t2 = t2pool.tile([OUTP, BS], f32, tag="t2")
                nc.vector.scalar_tensor_tensor(
                    t2[:], spk2_p[:], -0.5 * THR / SCODE, p2[:], ALU.mult, ALU.add
                )
                op2_eng.scalar_tensor_tensor(
                    u2[:], u2[:], BETA, t2[:], ALU.mult, ALU.add
                )
                spk2_new = spk2pool.tile([OUTP, BS], f16)
                emit_spike(spk2_new[:], u2[:], nthr2[:, 0:1], thr2p[:, 0:1])
                acc_eng = nc.gpsimd if ACC_GPS else nc.vector
                acc_eng.tensor_tensor(acc[:], acc[:], spk2_new[:], ALU.add)
                return spk2_new

            for t in range(T):
                xc_t = []
                xc0_t = []
                for b in range(XB):
                    if t == 0 and b == 0:
                        # fine-grained chunks so the first matmul only waits
                        # for 128KB, not 512KB
                        for c in range(XCHUNK):
                            x0 = xcpool.tile([128, BS], f16, tag="xc0")
                            nc.sync.dma_start(x0[:], xc0_d[c])
                            xc0_t.append(x0)
                        xc_t.append(None)
                        continue
                    eng = nc.sync if (b % 2 == 0) else nc.scalar
                    xt = xcpool.tile([128, XCHUNK * BS], f16, tag="xc")
                    eng.dma_start(xt[:], xc_d[t, b])
                    xc_t.append(xt)

                spk1_new = spk1pool.tile([128, MT1 * BS], f16)

                for m in range(MT1):
                    sl = slice(m * 128, (m + 1) * 128)
                    p1 = ps1.tile([128, BS], f32)
                    for k in range(KT1):
                        if xc_t[k // XCHUNK] is None:
                            rhs = xc0_t[k % XCHUNK][:]
                        else:
                            rhs = xc_t[k // XCHUNK][
                                :, (k % XCHUNK) * BS:(k % XCHUNK + 1) * BS]
                        nc.tensor.matmul(p1[:], w1h_t[k][:, sl], rhs,
                                         start=(k == 0), stop=(k == KT1 - 1))
                    if m == FC2_AFTER_M and t > 0:
                        # previous step's fc2 matmuls slot in behind this
                        # step's first fc1 block: their spike inputs are
                        # ready, so PE never stalls on the LIF chain.
                        p2_pending = emit_fc2_mms(spk1_prev)
                    msl = slice(m * BS, (m + 1) * BS)
                    t1 = t1pool.tile([128, BS], f32, tag="t1")
                    # t1 = p1 - (THR/2)*(spk+1) with the constant folded: the
                    # +-1 spike coding keeps the product exact in fp32.
                    nc.vector.scalar_tensor_tensor(
                        t1[:], spk1_prev[:, msl], -0.5 * THR / SCODE, p1[:],
                        ALU.mult, ALU.add
                    )
                    op2_eng.scalar_tensor_tensor(
                        u1[:, msl], u1[:, msl], BETA, t1[:], ALU.mult, ALU.add
                    )
                    emit_spike(spk1_new[:, msl], u1[:, msl], nthr1[:, m:m + 1],
                               thr1p[:, m:m + 1])
                if t > 0:
                    # lif2 ops stay at the m-loop tail so the DVE stream
                    # never head-of-line blocks on the fc2 psum.
                    spk2_prev = emit_lif2(p2_pending, spk2_prev)
                spk1_prev = spk1_new

            spk2_prev = emit_lif2(emit_fc2_mms(spk1_prev), spk2_prev)

            nc.sync.dma_start(out_d[:], acc[:])

    nc.compile()
    _NC_CACHE["nc"] = nc
    return nc


def prep_in_maps(spike_seq, W1, b1, W2, b2):
    x = np.asarray(spike_seq, dtype=np.float32)
    W1 = np.asarray(W1, dtype=np.float64)
    b1 = np.asarray(b1, dtype=np.float64)
    W2 = np.asarray(W2, dtype=np.float64)
    b2 = np.asarray(b2, dtype=np.float64)

    W1T32 = np.zeros((INP, H), np.float32)
    W1T32[:IN] = W1.T.astype(np.float32)
    w1h = W1T32.astype(np.float16)
    # +-SCODE spike coding: cur2 = (W2*0.5/SCODE) @ s + 0.5*sum(W2) (in C2)
    w2h = np.zeros((H, OUTP), np.float16)
    w2h[:, :OUT] = (W2.T.astype(np.float32) * (0.5 / SCODE)).astype(np.float16)

    # exact constant folds (float64): x centering, +-1 spikes, bias
    C1 = b1 + 0.5 * W1.sum(axis=1) - 0.5 * THR
    C2 = b2 + 0.5 * W2.sum(axis=1) - 0.5 * THR
    thr1 = (THR - C1 / (1.0 - BETA)).astype(np.float32).reshape(MT1, 128).T.copy()
    u1i = (-C1 / (1.0 - BETA)).astype(np.float32).reshape(MT1, 128).T.copy()
    thr2 = np.full((OUTP, 1), 1e30, np.float32)
    thr2[:OUT, 0] = (THR - C2 / (1.0 - BETA)).astype(np.float32)
    u2i = np.zeros((OUTP, 1), np.float32)
    u2i[:OUT, 0] = (-C2 / (1.0 - BETA)).astype(np.float32)

    common = dict(w1h=w1h, w2h=w2h, nthr1=-thr1, thr1p=thr1, u1i=u1i,
                  nthr2=-thr2, thr2p=thr2, u2i=u2i)

    xc_full = (x - 0.5).astype(np.float16)          # [T, B, IN]

    in_maps = []
    for c in range(NCORES):
        cs, ce = c * BS, (c + 1) * BS
        xc_c = np.zeros((T, INP, BS), np.float16)
        xc_c[:, :IN, :] = xc_full[:, cs:ce, :].transpose(0, 2, 1)
        # repack to [T, XB, 128, XCHUNK*BS]: DMA batch b holds k-chunks
        # XCHUNK*b..XCHUNK*(b+1)-1 side by side per partition row
        xc_c = (xc_c.reshape(T, XB, XCHUNK, 128, BS)
                .transpose(0, 1, 3, 2, 4)
                .reshape(T, XB, 128, XCHUNK * BS).copy())
        m = dict(common)
        m["xc"] = xc_c
        m["xc0"] = np.ascontiguousarray(
            xc_c[0, 0].reshape(128, XCHUNK, BS).transpose(1, 0, 2))
        in_maps.append(m)
    return in_maps


def gather_out(res):
    # +-SCODE spike coding: count = (sum/SCODE + T) / 2
    raw = np.concatenate(
        [res.results[c]["out"].T[:, :OUT] for c in range(NCORES)], axis=0
    ).astype(np.float32)
    return (raw / SCODE + float(T)) * 0.5


def kernel(spike_seq, W1, b1, W2, b2):
    nc = _build()
    in_maps = prep_in_maps(spike_seq, W1, b1, W2, b2)
    res = run_bass_kernel_spmd(nc, in_maps, core_ids=list(range(NCORES)))
    return gather_out(res)
